# revision 2
# baseline (speedup 1.0000x reference)
"""DilateAttention Trainium2 kernel (nn_DilateAttention).

Full inputs q,k,v: [8, 192, 56, 56] fp32. Output: [8, 56, 56, 192] fp32.
Sharded data-parallel over batch B=8 across 8 NeuronCores.

Per-core layout: channels-on-partitions, two head groups (heads 0-3 on 128
partitions, heads 4-5 on 64). Dilated-window shifts are pure free-dim AP
offsets into zero-padded k/v images. Cross-partition reductions (sum over
head_dim) and broadcasts (attn weights over head_dim) run on the PE via 0/1
selector matmuls; exp on ScalarE; elementwise mul/add on VectorE.
"""

import sys

for _p in ("/opt/trn_rl_repo",):
    if _p not in sys.path:
        sys.path.insert(0, _p)

import numpy as np

B = 8
C = 192
H = W = 56
HD = 32
NH = 6  # heads
KK = 9  # kernel*kernel
SCALE = HD ** -0.5
HWPIX = H * W  # 3136
PADH, PADW = 60, 64  # padded image in SBUF: rows y in [-2,58), cols x in [-4,60)
ROW0, COL0 = 2, 4  # offsets of y=0, x=0 inside padded image
SHIFTS = [(di, dj) for di in (-2, 0, 2) for dj in (-2, 0, 2)]
NROWS = KK * NH  # 54 score rows, row m = j*NH + h

_GROUPS = [(0, 128), (1, 64)]  # (group id, partitions); group g covers heads 4g..


def _build_consts():
    """Selector constants, arranged partition-major as 2D [P, ...] arrays."""
    consts = {}
    for g, P in _GROUPS:
        # selA[g]: lhsT for score reduce: [P, 9, 54], 1 at [p, j, j*6 + g*4 + p//32]
        a = np.zeros((P, KK, NROWS), np.float32)
        for p in range(P):
            for j in range(KK):
                a[p, j, j * NH + g * 4 + p // HD] = 1.0
        consts[f"selA{g}"] = a.reshape(P, KK * NROWS)
        # selB[g]: lhsT for attn broadcast: [54, 9, P], 1 at [j*6 + g*4 + p//32, j, p]
        b = np.zeros((NROWS, KK, P), np.float32)
        for j in range(KK):
            for p in range(P):
                b[j * NH + g * 4 + p // HD, j, p] = 1.0
        consts[f"selB{g}"] = b.reshape(NROWS, KK * P)
    # selD: [54, 6] sum over j per head
    d = np.zeros((NROWS, NH), np.float32)
    for m in range(NROWS):
        d[m, m % NH] = 1.0
    consts["selD"] = d
    # selN: [6, 54] broadcast per-head value to all (j,h) rows
    n = np.zeros((NH, NROWS), np.float32)
    for m in range(NROWS):
        n[m % NH, m] = 1.0
    consts["selN"] = n
    return consts


def _pad_memset(nc, t, P):
    """Zero only the pad strips of a [P, PADH, PADW] tile."""
    nc.gpsimd.memset(t[:, 0:ROW0, :], 0.0)
    nc.gpsimd.memset(t[:, ROW0 + H :, :], 0.0)
    nc.gpsimd.memset(t[:, ROW0 : ROW0 + H, 0:COL0], 0.0)
    nc.gpsimd.memset(t[:, ROW0 : ROW0 + H, COL0 + W :], 0.0)


def build_module():
    import concourse.bacc as bacc
    import concourse.mybir as mybir
    import concourse.tile as tile

    fp32 = mybir.dt.float32
    AL = mybir.AluOpType

    nc = bacc.Bacc("TRN2", target_bir_lowering=False, debug=False, num_devices=B)

    q_d = nc.dram_tensor("q", [C, HWPIX], fp32, kind="ExternalInput")
    k_d = nc.dram_tensor("k", [C, H, W], fp32, kind="ExternalInput")
    v_d = nc.dram_tensor("v", [C, H, W], fp32, kind="ExternalInput")
    o_d = nc.dram_tensor("o", [HWPIX, C], fp32, kind="ExternalOutput")
    consts = _build_consts()
    c_d = {
        name: nc.dram_tensor(name, list(arr.shape), fp32, kind="ExternalInput")
        for name, arr in consts.items()
    }

    with tile.TileContext(nc) as tc:
        with (
            tc.tile_pool(name="io", bufs=2) as io_pool,
            tc.tile_pool(name="work", bufs=2) as work_pool,
            tc.tile_pool(name="small", bufs=1) as small_pool,
        ):
            # ---- constants to SBUF
            sel_sb = {}
            for name, arr in consts.items():
                t = small_pool.tile(list(arr.shape), fp32, tag=f"c_{name}")
                nc.sync.dma_start(t[:], c_d[name][:])
                sel_sb[name] = t

            # ---- load q and padded k
            q_sb, k_sb = {}, {}
            for g, P in _GROUPS:
                qt = io_pool.tile([P, H, W], fp32, tag="q")
                nc.sync.dma_start(
                    qt[:], q_d[g * 128 : g * 128 + P, :].rearrange("p (a b) -> p a b", a=H)
                )
                q_sb[g] = qt
                kt = io_pool.tile([P, PADH, PADW], fp32, tag="kv")
                _pad_memset(nc, kt, P)
                nc.sync.dma_start(
                    kt[:, ROW0 : ROW0 + H, COL0 : COL0 + W],
                    k_d[g * 128 : g * 128 + P, :, :],
                )
                k_sb[g] = kt

            E_sb = small_pool.tile([NROWS, HWPIX], fp32, tag="E")

            # ---- stage A: scores. S_ps[m=(j*6+h), px] = sum_d q * k_shift
            with tc.tile_pool(name="psS", bufs=1, space="PSUM") as psS_pool:
                S_ps = psS_pool.tile([NROWS, HWPIX], fp32, tag="S")
                first = True
                for g, P in _GROUPS:
                    selA = sel_sb[f"selA{g}"].rearrange("p (j m) -> p j m", j=KK)
                    for j, (di, dj) in enumerate(SHIFTS):
                        prod = work_pool.tile([P, H, W], fp32, tag="prod")
                        kv = k_sb[g][
                            :, ROW0 + di : ROW0 + di + H, COL0 + dj : COL0 + dj + W
                        ]
                        nc.vector.tensor_tensor(prod[:], q_sb[g][:], kv, AL.mult)
                        pflat = prod.rearrange("p a b -> p (a b)")
                        for n0 in range(0, HWPIX, 512):
                            n1 = min(n0 + 512, HWPIX)
                            nc.tensor.matmul(
                                S_ps[:, n0:n1],
                                selA[:, j, :],
                                pflat[:, n0:n1],
                                start=first,
                                stop=(g == 1 and j == KK - 1),
                            )
                        first = False

                # exp(scale * S), evacuating PSUM
                nc.scalar.activation(
                    E_sb[:], S_ps[:], mybir.ActivationFunctionType.Exp, scale=float(SCALE)
                )

            # ---- stage B: normalize E by sum over j (chunked through PSUM)
            R_sb = small_pool.tile([NH, HWPIX], fp32, tag="R")
            Rs_sb = small_pool.tile([NH, HWPIX], fp32, tag="Rs")
            CHB = 448  # 7 chunks of 448 = 3136
            with tc.tile_pool(name="psB", bufs=2, space="PSUM") as psB_pool:
                for n0 in range(0, HWPIX, CHB):
                    n1 = n0 + CHB
                    D_ps = psB_pool.tile([NH, CHB], fp32, tag="D")
                    nc.tensor.matmul(
                        D_ps[:], sel_sb["selD"][:], E_sb[:, n0:n1], start=True, stop=True
                    )
                    nc.vector.reciprocal_approx_accurate(
                        R_sb[:, n0:n1], D_ps[:], Rs_sb[:, n0:n1]
                    )
                    RB_ps = psB_pool.tile([NROWS, CHB], fp32, tag="RB")
                    nc.tensor.matmul(
                        RB_ps[:], sel_sb["selN"][:], R_sb[:, n0:n1], start=True, stop=True
                    )
                    nc.vector.tensor_tensor(E_sb[:, n0:n1], E_sb[:, n0:n1], RB_ps[:], AL.mult)

            # ---- load padded v (reuses k slots)
            v_sb = {}
            for g, P in _GROUPS:
                vt = io_pool.tile([P, PADH, PADW], fp32, tag="kv")
                _pad_memset(nc, vt, P)
                nc.sync.dma_start(
                    vt[:, ROW0 : ROW0 + H, COL0 : COL0 + W],
                    v_d[g * 128 : g * 128 + P, :, :],
                )
                v_sb[g] = vt

            # ---- stage C: out[(hl,d), px] = sum_j attn_bcast_j * v_shift_j
            HALF = HWPIX // 2  # 1568
            acc = {}
            with tc.tile_pool(name="psC", bufs=2, space="PSUM") as psC_pool:
                for g, P in _GROUPS:
                    selB = sel_sb[f"selB{g}"].rearrange("m (j p) -> m j p", j=KK)
                    acc_g = work_pool.tile([P, HWPIX], fp32, tag="acc")
                    for j, (di, dj) in enumerate(SHIFTS):
                        a_sb = work_pool.tile([P, HWPIX], fp32, tag="attn")
                        for h0 in (0, HALF):
                            ab_ps = psC_pool.tile([P, HALF], fp32, tag="AB")
                            for n0 in range(0, HALF, 512):
                                n1 = min(n0 + 512, HALF)
                                nc.tensor.matmul(
                                    ab_ps[:, n0:n1],
                                    selB[:, j, :],
                                    E_sb[:, h0 + n0 : h0 + n1],
                                    start=True,
                                    stop=True,
                                )
                            nc.scalar.copy(a_sb[:, h0 : h0 + HALF], ab_ps[:])
                        vv = v_sb[g][
                            :, ROW0 + di : ROW0 + di + H, COL0 + dj : COL0 + dj + W
                        ]
                        av = a_sb.rearrange("p (a b) -> p a b", a=H)
                        if j == 0:
                            nc.vector.tensor_tensor(
                                acc_g.rearrange("p (a b) -> p a b", a=H), av, vv, AL.mult
                            )
                        else:
                            prod = work_pool.tile([P, H, W], fp32, tag="prod")
                            nc.vector.tensor_tensor(prod[:], av, vv, AL.mult)
                            nc.vector.tensor_tensor(
                                acc_g[:], acc_g[:], prod.rearrange("p a b -> p (a b)"), AL.add
                            )
                    acc[g] = acc_g

            # ---- output: 32x32 block transpose + strided DMA to [px, C]
            for g, P in _GROUPS:
                t_sb = work_pool.tile([P, HWPIX], fp32, tag="attn")
                nc.vector.transpose(t_sb[:], acc[g][:])
                for bc in range(P // 32):
                    c0 = g * 128 + bc * 32
                    src = t_sb[bc * 32 : (bc + 1) * 32, :].rearrange(
                        "p (bp ci) -> p bp ci", ci=32
                    )
                    dst = o_d.ap().rearrange("(bp pi) c -> pi bp c", pi=32)[:, :, c0 : c0 + 32]
                    nc.sync.dma_start(dst, src)

    nc.compile()
    return nc, consts


_CACHE = {}


def _get_module():
    if "nc" not in _CACHE:
        _CACHE["nc"], _CACHE["consts"] = build_module()
    return _CACHE["nc"], _CACHE["consts"]


def make_in_maps(q, k, v, consts):
    in_maps = []
    for b in range(B):
        m = {
            "q": np.ascontiguousarray(q[b].reshape(C, HWPIX)),
            "k": np.ascontiguousarray(k[b].reshape(C, H, W)),
            "v": np.ascontiguousarray(v[b].reshape(C, H, W)),
        }
        m.update(consts)
        in_maps.append(m)
    return in_maps


def kernel(q: np.ndarray, k: np.ndarray, v: np.ndarray) -> np.ndarray:
    from concourse import bass_utils

    nc, consts = _get_module()
    in_maps = make_in_maps(np.asarray(q), np.asarray(k), np.asarray(v), consts)
    res = bass_utils.run_bass_kernel_spmd(nc, in_maps, core_ids=list(range(B)))
    out = np.stack([r["o"].reshape(H, W, C) for r in res.results])
    return out
